# revision 45
# baseline (speedup 1.0000x reference)
"""Trainium2 Bass kernel for nn_CHTransform (cylindrical-harmonics decomposition).

Math: ch[b,c,n,k,l] = dtheta*dz * sum_{r,t,z} vol[b,c,r,t,z]
                       * Wr[|n|,k,r] * e^{i n theta_t}/sqrt(2pi) * e^{i pi l z_z}/sqrt(2)

The angular basis is even (cos) / odd (sin) in n and the radial basis depends
only on |n|, so only m=|n| in 0..3 is needed: a combined host-precomputed basis
C1[rt, j] (16 cos-cols (m,k) + 12 sin-cols (m>=1,k), 28 total) contracts r and
t in one TensorE pass; the tiny z-contraction against the axial basis and the
+/-n complex unfold happen on host during the unshard (64 x 28 x 96 floats).

Precision: the volume is host-converted to fp8 E3M4 (native PE dtype, 1 B/elt,
4 mantissa bits) with FIRST-ORDER NOISE SHAPING along z: the host stage-2
projects z onto |l|<=5 of 96 modes, so (1-z^-1)-shaped quantization noise is
attenuated ~3x by the projection (worst mode keeps 2*sin(pi*5/96) = 0.33).
The basis is e3m4 too (per-column scaled by ~8/max into the normal range,
unscaled on host).  Measured end-to-end rel err 1.214e-2 < 2e-2 gate.  This
quarters HBM traffic vs the fp32 baseline (27 -> 6.75 MiB/core), moving the
bottleneck to the PE itself: 55296 moving rows @ 1 cyc/row @ 2.4 GHz = 23 us.

Device (per core: 8 of the 64 (b,c) pairs, data-parallel, no communication):
  - vol arrives as [8, 128, 6912] e3m4: partition p holds 72 consecutive
    rt-rows; K-tile j of the contraction lives at free columns j*96..(j+1)*96,
    i.e. rt = p*72 + j, with C1 host-permuted to match.
  - (b,c) are processed in 2 groups of 4: one matmul per K-tile j with
    lhsT = C1_j [128, 28] e3m4 (stationary) and a 3D moving operand
    [128 x 4bc x 96z] e3m4 (N=384) accumulating into one PSUM bank
    [28, 384] over all 72 j.
  - host pre-arranges the volume chunk-major so every chunk DMA is one
    fully-contiguous [128, 8*jc*96] transfer (128 descriptors, >=4.6 KB
    each); early chunks ride the sync HWDGE ring behind a tiny basis head,
    late chunks + basis rest ride the scalar ring concurrently.
  - 42 N=128 warmup matmuls on a zeroed tile span the DMA-latency window
    (engine boot ~7 us + first-chunk transfer+receipt ~5 us) so the HAM
    clock gate is at K=8/8 before the first real matmul: the 144-matmul
    stream then runs at the theoretical 163 ns/matmul (measured 23.6 us,
    zero cold matmuls).
  - the accumulation is split at j=36 into two PSUM halves per group; the
    first half's PSUM->SBUF copy + bf16 store + HBM receipt all hide under
    the second half's matmuls, leaving only ~4.8 us of exposed tail.

Measured: 40.8-42.9 us (median ~41.6) on 8 cores vs 104.5 us fp32 baseline,
with ~7 us framework preamble, ~5 us first-DMA latency (covered by warmup),
23.6-24.4 us PE stream, and ~4.5 us tail.  Exec time has +-1-2 us run
jitter and occasional chip-wide slow phases (PE at 2.0 GHz -> ~15% slower).
"""

import math

import numpy as np
import ml_dtypes

import concourse.bacc as bacc
import concourse.mybir as mybir
import concourse.tile as tile
from concourse.bass_utils import run_bass_kernel_spmd

# Problem constants (hardcoded per spec nn_CHTransform_43439299231904)
B, C, R, T, Z = 8, 8, 96, 96, 96
MAX_N, MAX_K, MAX_L = 3, 4, 5
R_SCALE = 1.0
N_CORES = 8
BC = B * C                   # 64 (b,c) pairs
BC_PER_CORE = BC // N_CORES  # 8
RT = R * T                   # 9216
P = 128                      # SBUF partitions
Q = RT // P                  # 72 rt-rows per partition = # of K-tiles
NJ = 28                      # stage-1 output columns: 16 cos (m,k) + 12 sin
NL = 22                      # host stage-2 columns: 11 cos l + 11 sin l
GRP = 4                      # (b,c) pairs per matmul group (N = GRP*Z = 384)
NGRP = BC_PER_CORE // GRP    # 2
CHUNKS = [16, 16, 16, 12, 8, 4]  # K-tiles per DMA chunk (all 8 bc per chunk;
# chunk boundaries are multiples of 4 so j-quads never straddle chunks).
# Host pre-arranges the volume chunk-major: each chunk is one contiguous
# [128, 8*jc*96] transfer.
C1_HEAD = 12                 # K-tiles of basis in the head (sync-ring) DMA
NQUAD = Q // 4               # 18 j-quads; quad q covers js 4q..4q+3, one
# per 32-col group of the PE array (col-tiled concurrent matmuls)

BESSEL_ZEROS = {0: [2.4048, 5.5201, 8.6537, 11.7915, 14.9309],
                1: [3.8317, 7.0156, 10.1735, 13.3237, 16.4706],
                2: [5.1356, 8.4172, 11.6198, 14.796, 18.0155],
                3: [6.3802, 9.761, 13.0152, 16.2235, 19.4094]}

VOL_DT = mybir.dt.float8e3   # E3M4: native PE dtype, 1 cyc/row
W_DT = mybir.dt.float8e3     # basis dtype: e3m4, per-column scaled (host
# unscales); halves the basis bytes that compete with the first chunks
NWARM = 42                   # PE warmup matmuls (N=128) during DMA latency
NP_VOL_DT = ml_dtypes.float8_e3m4
TRACE = False                # test harness sets True for NTFF profiling
LAST_RESULTS = None          # BassKernelResults of the most recent run


def _bessel_j(n, x):
    xs = np.maximum(x, 1e-12)
    if n == 0:
        small = np.abs(x) < 1.0
        med = (np.abs(x) >= 1.0) & (np.abs(x) < 5.0)
        sm = 1.0 - x ** 2 / 4.0 + x ** 4 / 64.0
        md = np.cos(x - np.pi / 4) / np.sqrt(xs)
        lg = np.sqrt(2.0 / (np.pi * xs)) * np.cos(x - np.pi / 4)
        return np.where(small, sm, np.where(med, md, lg))
    elif n == 1:
        small = np.abs(x) < 1.0
        med = (np.abs(x) >= 1.0) & (np.abs(x) < 5.0)
        sm = x / 2.0 - x ** 3 / 16.0
        md = np.sin(x - np.pi / 4) / np.sqrt(xs)
        lg = np.sqrt(2.0 / (np.pi * xs)) * np.cos(x - 3 * np.pi / 4)
        return np.where(small, sm, np.where(med, md, lg))
    else:
        logfact = sum(math.log(i) for i in range(1, n + 1))
        small = np.abs(x) < 0.1 * n
        sm = np.exp(n * np.log(xs / 2.0) - logfact)
        lg = np.sqrt(2.0 / (np.pi * xs)) * np.cos(x - (2 * n + 1) * np.pi / 4)
        return np.where(small, sm, lg)


def _make_basis():
    """C1_perm [128, Q*NJ] f32 and ax_cat [Z, NL] f32; dtheta*dz in ax_cat."""
    r = np.linspace(0.0, 1.0, R) * R_SCALE
    theta = np.linspace(0.0, 2 * math.pi, T)
    z = np.linspace(-1.0, 1.0, Z)
    dr = R_SCALE / (R - 1)
    dtheta = 2 * math.pi / T
    dz = 2.0 / (Z - 1)
    Wm = np.zeros((4, MAX_K, R))
    for m in range(4):
        for k in range(1, MAX_K + 1):
            r_nk = BESSEL_ZEROS[m][k - 1]
            J = _bessel_j(m, r_nk * r)
            ss = (T * Z) * np.sum((J * r * dr) ** 2)
            norm = 1.0 / np.sqrt(ss) if ss > 1e-6 else 0.0
            Wm[m, k - 1] = J * norm * r * dr
    ang_scale = 1.0 / math.sqrt(2 * math.pi)
    C1 = np.zeros((RT, NJ))
    for m in range(4):
        cosm = np.cos(m * theta) * ang_scale
        sinm = np.sin(m * theta) * ang_scale
        for k in range(MAX_K):
            C1[:, m * 4 + k] = (Wm[m, k][:, None] * cosm[None, :]).reshape(-1)
            if m >= 1:
                C1[:, 16 + (m - 1) * 4 + k] = (
                    Wm[m, k][:, None] * sinm[None, :]).reshape(-1)
    # permute rows to the [128, 6912] data layout: K-tile j holds rt = p*Q + j
    C1_perm = C1.reshape(P, Q, NJ).reshape(P, Q * NJ)
    l_vals = np.arange(-MAX_L, MAX_L + 1)
    ax_scale = (1.0 / math.sqrt(2)) * dtheta * dz
    ax_cat = np.zeros((Z, NL))
    for li, lv in enumerate(l_vals):
        ax_cat[:, li] = np.cos(math.pi * lv * z) * ax_scale
        ax_cat[:, 11 + li] = np.sin(math.pi * lv * z) * ax_scale
    return (np.ascontiguousarray(C1_perm, dtype=np.float32),
            np.ascontiguousarray(ax_cat, dtype=np.float32))


def _combine(out2):
    """out2 [..., 28, 22] f32 -> ch [..., 7, 4, 11] complex64 (the +/-n unfold)."""
    lead = out2.shape[:-2]
    E = out2[..., :16, :].reshape(*lead, 4, MAX_K, 2, 11)  # cos block, q=0 re / 1 im
    O = out2[..., 16:, :].reshape(*lead, 3, MAX_K, 2, 11)  # sin block, m=1..3
    ch = np.zeros((*lead, 2 * MAX_N + 1, MAX_K, 2 * MAX_L + 1), dtype=np.complex64)
    ch[..., 3, :, :] = E[..., 0, :, 0, :] + 1j * E[..., 0, :, 1, :]
    for m in range(1, 4):
        Er, Ei = E[..., m, :, 0, :], E[..., m, :, 1, :]
        Or_, Oi = O[..., m - 1, :, 0, :], O[..., m - 1, :, 1, :]
        ch[..., 3 + m, :, :] = (Er - Oi) + 1j * (Ei + Or_)
        ch[..., 3 - m, :, :] = (Er + Oi) + 1j * (Ei - Or_)
    return ch


def _build_nc():
    f32 = mybir.dt.float32
    nc = bacc.Bacc("TRN2", target_bir_lowering=False, debug=False,
                   num_devices=N_CORES)
    vol_in = nc.dram_tensor("vol", [P, BC_PER_CORE * Q * Z], VOL_DT,
                            kind="ExternalInput")
    c1_in = nc.dram_tensor("c1", [P, NQUAD * P], W_DT, kind="ExternalInput")
    out = nc.dram_tensor("out", [P, NGRP * GRP * Z], mybir.dt.bfloat16,
                         kind="ExternalOutput")

    with tile.TileContext(nc) as tc:
        with (
            tc.tile_pool(name="consts", bufs=1) as consts,
            tc.tile_pool(name="vpool", bufs=5) as vpool,
            tc.tile_pool(name="obuf", bufs=2) as obuf,
            tc.tile_pool(name="pspool", bufs=1, space="PSUM") as pspool,
        ):
            # basis head (weights for the first chunk's K-tiles) rides the
            # sync ring FIRST: it is tiny, so the first matmul is gated only
            # by the first volume chunk.  The scalar ring pays a ~1.3 us
            # ACT_TABLE_LOAD before its first instruction, so it only gets
            # late chunks + the second output copy.
            c1_sb = consts.tile([P, NQUAD * P], W_DT)
            hq = (C1_HEAD // 4) * P
            nc.sync.dma_start(c1_sb[:, :hq], c1_in[:, :hq])
            # rest of the basis on the scalar ring: flows concurrently with
            # the early chunks, lands well before K-tile C1_HEAD is due
            nc.scalar.dma_start(c1_sb[:, hq:], c1_in[:, hq:])
            # PE warmup: ~40 tiny matmuls on a zeroed tile keep the PE busy
            # during the DMA-latency window so HAM un-throttles (K=8/8)
            # before the first real matmul; results land in a scratch bank.
            wz = consts.tile([P, 512], VOL_DT)
            nc.vector.memset(wz[:], 0)
            psw = pspool.tile([NJ, 128], f32, name="psw")
            for _ in range(NWARM):
                nc.tensor.matmul(psw[:], wz[:, :NJ], wz[:, 128:256],
                                 start=True, stop=True)
            # one full [128, 384] accumulator per bc-group: col-group cg
            # holds partials for js == cg (mod 4) at partitions 32cg..32cg+27
            acc = [pspool.tile([P, GRP * Z], f32, name=f"acc{g}")
                   for g in range(NGRP)]
            j0 = 0
            off = 0
            for ci, jchunk in enumerate(CHUNKS):
                cb = BC_PER_CORE * jchunk * Z
                v8 = vpool.tile([P, BC_PER_CORE * max(CHUNKS) * Z], VOL_DT,
                                padded_shape=[P, BC_PER_CORE * max(CHUNKS) * Z])
                # early chunks on sync (behind the tiny c1 head); late
                # chunks ride the scalar ring with the basis rest
                eng = nc.sync if ci % 2 == 0 else nc.scalar
                eng.dma_start(v8[:, :cb], vol_in[:, off:off + cb])
                off += cb
                v8r = v8[:, :cb].rearrange(
                    "p (b j z) -> p b j z", b=BC_PER_CORE, j=jchunk)
                for jj in range(jchunk):
                    j = j0 + jj
                    q, cg = divmod(j, 4)
                    for g in range(NGRP):
                        # col-tiled: consecutive js occupy different 32-col
                        # groups of the array and run concurrently
                        nc.tensor.matmul(
                            acc[g][32 * cg:32 * cg + NJ, :],
                            c1_sb[:, q * P + 32 * cg:q * P + 32 * cg + NJ],
                            v8r[:, g * GRP:(g + 1) * GRP, jj, :],
                            start=(q == 0),
                            stop=(q == NQUAD - 1),
                            tile_position=(0, 32 * cg),
                            skip_group_check=True,
                        )
                j0 += jchunk
            # full accumulators out; the 4-way col-group add happens on host
            ob = obuf.tile([P, NGRP * GRP * Z], mybir.dt.bfloat16)
            nc.vector.tensor_copy(ob[:, :GRP * Z], acc[0][:])
            nc.scalar.copy(ob[:, GRP * Z:], acc[1][:])
            nc.sync.dma_start(out[:], ob[:])

    nc.compile()
    return nc


_NC_CACHE = None


def _get_nc():
    global _NC_CACHE
    if _NC_CACHE is None:
        _NC_CACHE = _build_nc()
    return _NC_CACHE


def kernel(cylindrical_volume):
    global LAST_RESULTS
    vol = np.asarray(cylindrical_volume, dtype=np.float32)
    assert vol.shape == (B, C, R, T, Z), vol.shape
    c1_perm, ax_cat = _make_basis()
    # basis -> e3m4 with per-output-column scale (values are ~1e-4, far
    # below e3m4's normal range; scale to ~8 and unscale on host)
    C1 = c1_perm.reshape(P, Q, NJ)
    lam = 8.0 / np.abs(C1).max(axis=(0, 1))               # [NJ]
    c1q = np.zeros((P, Q // 4, 4, 32), dtype=NP_VOL_DT)
    c1q[:, :, :, :NJ] = (C1 * lam).reshape(P, Q // 4, 4, NJ).astype(NP_VOL_DT)
    c1_dev = np.ascontiguousarray(c1q.reshape(P, (Q // 4) * P))
    # volume -> e3m4 with first-order noise shaping along z: the host
    # stage-2 projects z onto |l|<=5 of 96 modes, so (1-z^-1)-shaped
    # quantization noise is filtered ~3x (rel err 1.4e-2 -> 4e-3 at fp16
    # basis; 1.2e-2 with the e3m4 basis)
    vr = np.ascontiguousarray(vol).reshape(BC, P, Q, Z)
    volq = np.empty((BC, P, Q, Z), dtype=NP_VOL_DT)
    err = np.zeros((BC, P, Q), dtype=np.float32)
    for k in range(Z):
        acc = vr[..., k] + err
        qk = acc.astype(NP_VOL_DT)
        volq[..., k] = qk
        err = acc - qk.astype(np.float32)

    nc = _get_nc()
    in_maps = []
    for i in range(N_CORES):
        vc = volq[i * BC_PER_CORE:(i + 1) * BC_PER_CORE]  # [8, 128, 72, 96]
        vt = vc.transpose(1, 0, 2, 3)                     # [128, 8, 72, 96]
        j0 = 0
        blocks = []
        for jc in CHUNKS:
            blocks.append(vt[:, :, j0:j0 + jc, :].reshape(P, -1))
            j0 += jc
        in_maps.append({"vol": np.ascontiguousarray(np.concatenate(blocks, axis=1)),
                        "c1": c1_dev})
    import os
    try:
        res = run_bass_kernel_spmd(nc, in_maps, list(range(N_CORES)),
                                   trace=TRACE)
    except ModuleNotFoundError:
        # BASS_TRACE set but this image lacks the axon NTFF hook module;
        # rerun without tracing rather than failing
        os.environ["BASS_NEVER_TRACE"] = "1"
        try:
            res = run_bass_kernel_spmd(nc, in_maps, list(range(N_CORES)),
                                       trace=False)
        finally:
            os.environ.pop("BASS_NEVER_TRACE", None)
    LAST_RESULTS = res
    # per-core out [128, NGRP*GRP*96]: sum the 4 col-group partials
    # (rows 32cg+c) -> [8bc, 28, 96z]
    S = np.concatenate(
        [res.results[i]["out"].astype(np.float32)
         .reshape(4, 32, NGRP, GRP, Z)[:, :NJ].sum(axis=0)
         .transpose(1, 2, 0, 3).reshape(BC_PER_CORE, NJ, Z)
         for i in range(N_CORES)], axis=0)          # [64, 28, 96]
    S /= lam[None, :, None]                          # undo basis column scale
    out2 = np.einsum('bjz,zl->bjl', S, ax_cat)       # host stage 2: [64, 28, 22]
    ch = _combine(out2)
    return ch.reshape(B, C, 2 * MAX_N + 1, MAX_K, 2 * MAX_L + 1)


# revision 46
# speedup vs baseline: 1.0628x; 1.0628x over previous
"""Trainium2 Bass kernel for nn_CHTransform (cylindrical-harmonics decomposition).

Math: ch[b,c,n,k,l] = dtheta*dz * sum_{r,t,z} vol[b,c,r,t,z]
                       * Wr[|n|,k,r] * e^{i n theta_t}/sqrt(2pi) * e^{i pi l z_z}/sqrt(2)

The angular basis is even (cos) / odd (sin) in n and the radial basis depends
only on |n|, so only m=|n| in 0..3 is needed: a combined host-precomputed basis
C1[rt, j] (16 cos-cols (m,k) + 12 sin-cols (m>=1,k), 28 total) contracts r and
t in one TensorE pass; the tiny z-contraction against the axial basis and the
+/-n complex unfold happen on host during the unshard (64 x 28 x 96 floats).

Precision: the volume is host-converted to fp8 E3M4 (native PE dtype, 1 B/elt,
4 mantissa bits) with FIRST-ORDER NOISE SHAPING along z: the host stage-2
projects z onto |l|<=5 of 96 modes, so (1-z^-1)-shaped quantization noise is
attenuated ~3x by the projection (worst mode keeps 2*sin(pi*5/96) = 0.33).
The basis is e3m4 too (per-column scaled by ~8/max into the normal range,
unscaled on host).  Measured end-to-end rel err 1.214e-2 < 2e-2 gate.  This
quarters HBM traffic vs the fp32 baseline (27 -> 6.75 MiB/core), moving the
bottleneck to the PE itself: 55296 moving rows @ 1 cyc/row @ 2.4 GHz = 23 us.

Device (per core: 8 of the 64 (b,c) pairs, data-parallel, no communication):
  - vol arrives as [8, 128, 6912] e3m4: partition p holds 72 consecutive
    rt-rows; K-tile j of the contraction lives at free columns j*96..(j+1)*96,
    i.e. rt = p*72 + j, with C1 host-permuted to match.
  - (b,c) are processed in 2 groups of 4.  The 28-col stationary uses only
    28/128 PE columns, so K-tiles are COL-TILED: quad q's four js occupy
    the four 32-col groups (tile_position=(0,32cg)) and their matmuls run
    CONCURRENTLY in the array (~3.7x stream speedup, PE now sub-critical).
    Each col-group accumulates js == cg (mod 4) into partition slice
    [32cg:32cg+28] of a full [128, 384] PSUM bank per group; the 4-way
    cross-partition add happens on host (output is [128, 768] bf16).
  - host pre-arranges the volume chunk-major so every chunk DMA is one
    fully-contiguous [128, 8*jc*96] transfer (128 descriptors, >=4.6 KB
    each); early chunks ride the sync HWDGE ring behind a tiny basis head,
    late chunks + basis rest ride the scalar ring concurrently.
  - 42 N=128 warmup matmuls on a zeroed tile span the DMA-latency window
    so the HAM clock gate is at K=8/8 when real matmuls start.
  - with the PE sub-critical the kernel is DMA-bound end-to-end: chunks
    are front-loaded big ([16,16,16,12,8,4]) for maximum flow rate, the
    last chunk small so its completion sem (+~2.9 us receipt lag) lands
    right after flow-end.

Measured: 34.7-34.8 us (full clock) on 8 cores vs 104.5 us fp32 baseline:
~7 us framework preamble, ~18 us HBM flow @ ~400 GB/s, last-chunk sem
lag, ~4 us tail.  Exec time has +-1-2 us run jitter and occasional
chip-wide slow phases (PE at 2.0 GHz, DMA slower too -> ~41 us).
"""

import math

import numpy as np
import ml_dtypes

import concourse.bacc as bacc
import concourse.mybir as mybir
import concourse.tile as tile
from concourse.bass_utils import run_bass_kernel_spmd

# Problem constants (hardcoded per spec nn_CHTransform_43439299231904)
B, C, R, T, Z = 8, 8, 96, 96, 96
MAX_N, MAX_K, MAX_L = 3, 4, 5
R_SCALE = 1.0
N_CORES = 8
BC = B * C                   # 64 (b,c) pairs
BC_PER_CORE = BC // N_CORES  # 8
RT = R * T                   # 9216
P = 128                      # SBUF partitions
Q = RT // P                  # 72 rt-rows per partition = # of K-tiles
NJ = 28                      # stage-1 output columns: 16 cos (m,k) + 12 sin
NL = 22                      # host stage-2 columns: 11 cos l + 11 sin l
GRP = 4                      # (b,c) pairs per matmul group (N = GRP*Z = 384)
NGRP = BC_PER_CORE // GRP    # 2
CHUNKS = [16, 16, 16, 12, 8, 4]  # K-tiles per DMA chunk (all 8 bc per chunk;
# chunk boundaries are multiples of 4 so j-quads never straddle chunks).
# Host pre-arranges the volume chunk-major: each chunk is one contiguous
# [128, 8*jc*96] transfer.
C1_HEAD = 12                 # K-tiles of basis in the head (sync-ring) DMA
NQUAD = Q // 4               # 18 j-quads; quad q covers js 4q..4q+3, one
# per 32-col group of the PE array (col-tiled concurrent matmuls)

BESSEL_ZEROS = {0: [2.4048, 5.5201, 8.6537, 11.7915, 14.9309],
                1: [3.8317, 7.0156, 10.1735, 13.3237, 16.4706],
                2: [5.1356, 8.4172, 11.6198, 14.796, 18.0155],
                3: [6.3802, 9.761, 13.0152, 16.2235, 19.4094]}

VOL_DT = mybir.dt.float8e3   # E3M4: native PE dtype, 1 cyc/row
W_DT = mybir.dt.float8e3     # basis dtype: e3m4, per-column scaled (host
# unscales); halves the basis bytes that compete with the first chunks
NWARM = 42                   # PE warmup matmuls (N=128) during DMA latency
NP_VOL_DT = ml_dtypes.float8_e3m4
TRACE = False                # test harness sets True for NTFF profiling
LAST_RESULTS = None          # BassKernelResults of the most recent run


def _bessel_j(n, x):
    xs = np.maximum(x, 1e-12)
    if n == 0:
        small = np.abs(x) < 1.0
        med = (np.abs(x) >= 1.0) & (np.abs(x) < 5.0)
        sm = 1.0 - x ** 2 / 4.0 + x ** 4 / 64.0
        md = np.cos(x - np.pi / 4) / np.sqrt(xs)
        lg = np.sqrt(2.0 / (np.pi * xs)) * np.cos(x - np.pi / 4)
        return np.where(small, sm, np.where(med, md, lg))
    elif n == 1:
        small = np.abs(x) < 1.0
        med = (np.abs(x) >= 1.0) & (np.abs(x) < 5.0)
        sm = x / 2.0 - x ** 3 / 16.0
        md = np.sin(x - np.pi / 4) / np.sqrt(xs)
        lg = np.sqrt(2.0 / (np.pi * xs)) * np.cos(x - 3 * np.pi / 4)
        return np.where(small, sm, np.where(med, md, lg))
    else:
        logfact = sum(math.log(i) for i in range(1, n + 1))
        small = np.abs(x) < 0.1 * n
        sm = np.exp(n * np.log(xs / 2.0) - logfact)
        lg = np.sqrt(2.0 / (np.pi * xs)) * np.cos(x - (2 * n + 1) * np.pi / 4)
        return np.where(small, sm, lg)


def _make_basis():
    """C1_perm [128, Q*NJ] f32 and ax_cat [Z, NL] f32; dtheta*dz in ax_cat."""
    r = np.linspace(0.0, 1.0, R) * R_SCALE
    theta = np.linspace(0.0, 2 * math.pi, T)
    z = np.linspace(-1.0, 1.0, Z)
    dr = R_SCALE / (R - 1)
    dtheta = 2 * math.pi / T
    dz = 2.0 / (Z - 1)
    Wm = np.zeros((4, MAX_K, R))
    for m in range(4):
        for k in range(1, MAX_K + 1):
            r_nk = BESSEL_ZEROS[m][k - 1]
            J = _bessel_j(m, r_nk * r)
            ss = (T * Z) * np.sum((J * r * dr) ** 2)
            norm = 1.0 / np.sqrt(ss) if ss > 1e-6 else 0.0
            Wm[m, k - 1] = J * norm * r * dr
    ang_scale = 1.0 / math.sqrt(2 * math.pi)
    C1 = np.zeros((RT, NJ))
    for m in range(4):
        cosm = np.cos(m * theta) * ang_scale
        sinm = np.sin(m * theta) * ang_scale
        for k in range(MAX_K):
            C1[:, m * 4 + k] = (Wm[m, k][:, None] * cosm[None, :]).reshape(-1)
            if m >= 1:
                C1[:, 16 + (m - 1) * 4 + k] = (
                    Wm[m, k][:, None] * sinm[None, :]).reshape(-1)
    # permute rows to the [128, 6912] data layout: K-tile j holds rt = p*Q + j
    C1_perm = C1.reshape(P, Q, NJ).reshape(P, Q * NJ)
    l_vals = np.arange(-MAX_L, MAX_L + 1)
    ax_scale = (1.0 / math.sqrt(2)) * dtheta * dz
    ax_cat = np.zeros((Z, NL))
    for li, lv in enumerate(l_vals):
        ax_cat[:, li] = np.cos(math.pi * lv * z) * ax_scale
        ax_cat[:, 11 + li] = np.sin(math.pi * lv * z) * ax_scale
    return (np.ascontiguousarray(C1_perm, dtype=np.float32),
            np.ascontiguousarray(ax_cat, dtype=np.float32))


def _combine(out2):
    """out2 [..., 28, 22] f32 -> ch [..., 7, 4, 11] complex64 (the +/-n unfold)."""
    lead = out2.shape[:-2]
    E = out2[..., :16, :].reshape(*lead, 4, MAX_K, 2, 11)  # cos block, q=0 re / 1 im
    O = out2[..., 16:, :].reshape(*lead, 3, MAX_K, 2, 11)  # sin block, m=1..3
    ch = np.zeros((*lead, 2 * MAX_N + 1, MAX_K, 2 * MAX_L + 1), dtype=np.complex64)
    ch[..., 3, :, :] = E[..., 0, :, 0, :] + 1j * E[..., 0, :, 1, :]
    for m in range(1, 4):
        Er, Ei = E[..., m, :, 0, :], E[..., m, :, 1, :]
        Or_, Oi = O[..., m - 1, :, 0, :], O[..., m - 1, :, 1, :]
        ch[..., 3 + m, :, :] = (Er - Oi) + 1j * (Ei + Or_)
        ch[..., 3 - m, :, :] = (Er + Oi) + 1j * (Ei - Or_)
    return ch


def _build_nc():
    f32 = mybir.dt.float32
    nc = bacc.Bacc("TRN2", target_bir_lowering=False, debug=False,
                   num_devices=N_CORES)
    vol_in = nc.dram_tensor("vol", [P, BC_PER_CORE * Q * Z], VOL_DT,
                            kind="ExternalInput")
    c1_in = nc.dram_tensor("c1", [P, NQUAD * P], W_DT, kind="ExternalInput")
    out = nc.dram_tensor("out", [P, NGRP * GRP * Z], mybir.dt.bfloat16,
                         kind="ExternalOutput")

    with tile.TileContext(nc) as tc:
        with (
            tc.tile_pool(name="consts", bufs=1) as consts,
            tc.tile_pool(name="vpool", bufs=5) as vpool,
            tc.tile_pool(name="obuf", bufs=2) as obuf,
            tc.tile_pool(name="pspool", bufs=1, space="PSUM") as pspool,
        ):
            # basis head (weights for the first chunk's K-tiles) rides the
            # sync ring FIRST: it is tiny, so the first matmul is gated only
            # by the first volume chunk.  The scalar ring pays a ~1.3 us
            # ACT_TABLE_LOAD before its first instruction, so it only gets
            # late chunks + the second output copy.
            c1_sb = consts.tile([P, NQUAD * P], W_DT)
            hq = (C1_HEAD // 4) * P
            nc.sync.dma_start(c1_sb[:, :hq], c1_in[:, :hq])
            # rest of the basis on the scalar ring: flows concurrently with
            # the early chunks, lands well before K-tile C1_HEAD is due
            nc.scalar.dma_start(c1_sb[:, hq:], c1_in[:, hq:])
            # PE warmup: ~40 tiny matmuls on a zeroed tile keep the PE busy
            # during the DMA-latency window so HAM un-throttles (K=8/8)
            # before the first real matmul; results land in a scratch bank.
            wz = consts.tile([P, 512], VOL_DT)
            nc.vector.memset(wz[:], 0)
            psw = pspool.tile([NJ, 128], f32, name="psw")
            for _ in range(NWARM):
                nc.tensor.matmul(psw[:], wz[:, :NJ], wz[:, 128:256],
                                 start=True, stop=True)
            # one full [128, 384] accumulator per bc-group: col-group cg
            # holds partials for js == cg (mod 4) at partitions 32cg..32cg+27
            acc = [pspool.tile([P, GRP * Z], f32, name=f"acc{g}")
                   for g in range(NGRP)]
            j0 = 0
            off = 0
            for ci, jchunk in enumerate(CHUNKS):
                cb = BC_PER_CORE * jchunk * Z
                v8 = vpool.tile([P, BC_PER_CORE * max(CHUNKS) * Z], VOL_DT,
                                padded_shape=[P, BC_PER_CORE * max(CHUNKS) * Z])
                # early chunks on sync (behind the tiny c1 head); late
                # chunks ride the scalar ring with the basis rest
                eng = nc.sync if ci % 2 == 0 else nc.scalar
                eng.dma_start(v8[:, :cb], vol_in[:, off:off + cb])
                off += cb
                v8r = v8[:, :cb].rearrange(
                    "p (b j z) -> p b j z", b=BC_PER_CORE, j=jchunk)
                for jj in range(jchunk):
                    j = j0 + jj
                    q, cg = divmod(j, 4)
                    for g in range(NGRP):
                        # col-tiled: consecutive js occupy different 32-col
                        # groups of the array and run concurrently
                        nc.tensor.matmul(
                            acc[g][32 * cg:32 * cg + NJ, :],
                            c1_sb[:, q * P + 32 * cg:q * P + 32 * cg + NJ],
                            v8r[:, g * GRP:(g + 1) * GRP, jj, :],
                            start=(q == 0),
                            stop=(q == NQUAD - 1),
                            tile_position=(0, 32 * cg),
                            skip_group_check=True,
                        )
                j0 += jchunk
            # full accumulators out; the 4-way col-group add happens on host
            ob = obuf.tile([P, NGRP * GRP * Z], mybir.dt.bfloat16)
            nc.vector.tensor_copy(ob[:, :GRP * Z], acc[0][:])
            nc.scalar.copy(ob[:, GRP * Z:], acc[1][:])
            nc.sync.dma_start(out[:], ob[:])

    nc.compile()
    return nc


_NC_CACHE = None


def _get_nc():
    global _NC_CACHE
    if _NC_CACHE is None:
        _NC_CACHE = _build_nc()
    return _NC_CACHE


def kernel(cylindrical_volume):
    global LAST_RESULTS
    vol = np.asarray(cylindrical_volume, dtype=np.float32)
    assert vol.shape == (B, C, R, T, Z), vol.shape
    c1_perm, ax_cat = _make_basis()
    # basis -> e3m4 with per-output-column scale (values are ~1e-4, far
    # below e3m4's normal range; scale to ~8 and unscale on host)
    C1 = c1_perm.reshape(P, Q, NJ)
    lam = 8.0 / np.abs(C1).max(axis=(0, 1))               # [NJ]
    c1q = np.zeros((P, Q // 4, 4, 32), dtype=NP_VOL_DT)
    c1q[:, :, :, :NJ] = (C1 * lam).reshape(P, Q // 4, 4, NJ).astype(NP_VOL_DT)
    c1_dev = np.ascontiguousarray(c1q.reshape(P, (Q // 4) * P))
    # volume -> e3m4 with first-order noise shaping along z: the host
    # stage-2 projects z onto |l|<=5 of 96 modes, so (1-z^-1)-shaped
    # quantization noise is filtered ~3x (rel err 1.4e-2 -> 4e-3 at fp16
    # basis; 1.2e-2 with the e3m4 basis)
    vr = np.ascontiguousarray(vol).reshape(BC, P, Q, Z)
    volq = np.empty((BC, P, Q, Z), dtype=NP_VOL_DT)
    err = np.zeros((BC, P, Q), dtype=np.float32)
    for k in range(Z):
        acc = vr[..., k] + err
        qk = acc.astype(NP_VOL_DT)
        volq[..., k] = qk
        err = acc - qk.astype(np.float32)

    nc = _get_nc()
    in_maps = []
    for i in range(N_CORES):
        vc = volq[i * BC_PER_CORE:(i + 1) * BC_PER_CORE]  # [8, 128, 72, 96]
        vt = vc.transpose(1, 0, 2, 3)                     # [128, 8, 72, 96]
        j0 = 0
        blocks = []
        for jc in CHUNKS:
            blocks.append(vt[:, :, j0:j0 + jc, :].reshape(P, -1))
            j0 += jc
        in_maps.append({"vol": np.ascontiguousarray(np.concatenate(blocks, axis=1)),
                        "c1": c1_dev})
    import os
    try:
        res = run_bass_kernel_spmd(nc, in_maps, list(range(N_CORES)),
                                   trace=TRACE)
    except ModuleNotFoundError:
        # BASS_TRACE set but this image lacks the axon NTFF hook module;
        # rerun without tracing rather than failing
        os.environ["BASS_NEVER_TRACE"] = "1"
        try:
            res = run_bass_kernel_spmd(nc, in_maps, list(range(N_CORES)),
                                       trace=False)
        finally:
            os.environ.pop("BASS_NEVER_TRACE", None)
    LAST_RESULTS = res
    # per-core out [128, NGRP*GRP*96]: sum the 4 col-group partials
    # (rows 32cg+c) -> [8bc, 28, 96z]
    S = np.concatenate(
        [res.results[i]["out"].astype(np.float32)
         .reshape(4, 32, NGRP, GRP, Z)[:, :NJ].sum(axis=0)
         .transpose(1, 2, 0, 3).reshape(BC_PER_CORE, NJ, Z)
         for i in range(N_CORES)], axis=0)          # [64, 28, 96]
    S /= lam[None, :, None]                          # undo basis column scale
    out2 = np.einsum('bjz,zl->bjl', S, ax_cat)       # host stage 2: [64, 28, 22]
    ch = _combine(out2)
    return ch.reshape(B, C, 2 * MAX_N + 1, MAX_K, 2 * MAX_L + 1)


# revision 47
# speedup vs baseline: 1.0697x; 1.0065x over previous
"""Trainium2 Bass kernel for nn_CHTransform (cylindrical-harmonics decomposition).

Math: ch[b,c,n,k,l] = dtheta*dz * sum_{r,t,z} vol[b,c,r,t,z]
                       * Wr[|n|,k,r] * e^{i n theta_t}/sqrt(2pi) * e^{i pi l z_z}/sqrt(2)

The angular basis is even (cos) / odd (sin) in n and the radial basis depends
only on |n|, so only m=|n| in 0..3 is needed: a combined host-precomputed basis
C1[rt, j] (16 cos-cols (m,k) + 12 sin-cols (m>=1,k), 28 total) contracts r and
t in one TensorE pass; the tiny z-contraction against the axial basis and the
+/-n complex unfold happen on host during the unshard (64 x 28 x 96 floats).

Precision: the volume is host-converted to fp8 E3M4 (native PE dtype, 1 B/elt,
4 mantissa bits) with FIRST-ORDER NOISE SHAPING along z: the host stage-2
projects z onto |l|<=5 of 96 modes, so (1-z^-1)-shaped quantization noise is
attenuated ~3x by the projection (worst mode keeps 2*sin(pi*5/96) = 0.33).
The basis is e3m4 too (per-column scaled by ~8/max into the normal range,
unscaled on host).  Measured end-to-end rel err 1.214e-2 < 2e-2 gate.  This
quarters HBM traffic vs the fp32 baseline (27 -> 6.75 MiB/core), moving the
bottleneck to the PE itself: 55296 moving rows @ 1 cyc/row @ 2.4 GHz = 23 us.

Device (per core: 8 of the 64 (b,c) pairs, data-parallel, no communication):
  - vol arrives as [8, 128, 6912] e3m4: partition p holds 72 consecutive
    rt-rows; K-tile j of the contraction lives at free columns j*96..(j+1)*96,
    i.e. rt = p*72 + j, with C1 host-permuted to match.
  - (b,c) are processed in 2 groups of 4.  The 28-col stationary uses only
    28/128 PE columns, so K-tiles are COL-TILED: quad q's four js occupy
    the four 32-col groups (tile_position=(0,32cg)) and their matmuls run
    CONCURRENTLY in the array (~3.7x stream speedup, PE now sub-critical).
    Each col-group accumulates js == cg (mod 4) into partition slice
    [32cg:32cg+28] of a full [128, 384] PSUM bank per group; the 4-way
    cross-partition add happens on host (output is [128, 768] bf16).
  - host pre-arranges the volume chunk-major so every chunk DMA is one
    fully-contiguous [128, 8*jc*96] transfer (128 descriptors, >=4.6 KB
    each); early chunks ride the sync HWDGE ring behind a tiny basis head,
    late chunks + basis rest ride the scalar ring concurrently.
  - 42 N=128 warmup matmuls on a zeroed tile span the DMA-latency window
    so the HAM clock gate is at K=8/8 when real matmuls start.
  - with the PE sub-critical the kernel is DMA-bound end-to-end: chunks
    are front-loaded big ([16,16,16,12,8,4]) for maximum flow rate, the
    last chunk small so its completion sem (+~2.9 us receipt lag) lands
    right after flow-end.

Measured: 34.7-34.8 us (full clock) on 8 cores vs 104.5 us fp32 baseline:
~7 us framework preamble, ~18 us HBM flow @ ~400 GB/s, last-chunk sem
lag, ~4 us tail.  Exec time has +-1-2 us run jitter and occasional
chip-wide slow phases (PE at 2.0 GHz, DMA slower too -> ~41 us).
"""

import math

import numpy as np
import ml_dtypes

import concourse.bacc as bacc
import concourse.mybir as mybir
import concourse.tile as tile
from concourse.bass_utils import run_bass_kernel_spmd

# Problem constants (hardcoded per spec nn_CHTransform_43439299231904)
B, C, R, T, Z = 8, 8, 96, 96, 96
MAX_N, MAX_K, MAX_L = 3, 4, 5
R_SCALE = 1.0
N_CORES = 8
BC = B * C                   # 64 (b,c) pairs
BC_PER_CORE = BC // N_CORES  # 8
RT = R * T                   # 9216
P = 128                      # SBUF partitions
Q = RT // P                  # 72 rt-rows per partition = # of K-tiles
NJ = 28                      # stage-1 output columns: 16 cos (m,k) + 12 sin
NL = 22                      # host stage-2 columns: 11 cos l + 11 sin l
GRP = 4                      # (b,c) pairs per matmul group (N = GRP*Z = 384)
NGRP = BC_PER_CORE // GRP    # 2
CHUNKS = [12, 16, 12, 12, 8, 8, 4]  # K-tiles per DMA chunk (all 8 bc per chunk;
# chunk boundaries are multiples of 4 so j-quads never straddle chunks).
# Host pre-arranges the volume chunk-major: each chunk is one contiguous
# [128, 8*jc*96] transfer.
C1_HEAD = 12                 # K-tiles of basis in the head (sync-ring) DMA
NQUAD = Q // 4               # 18 j-quads; quad q covers js 4q..4q+3, one
# per 32-col group of the PE array (col-tiled concurrent matmuls)

BESSEL_ZEROS = {0: [2.4048, 5.5201, 8.6537, 11.7915, 14.9309],
                1: [3.8317, 7.0156, 10.1735, 13.3237, 16.4706],
                2: [5.1356, 8.4172, 11.6198, 14.796, 18.0155],
                3: [6.3802, 9.761, 13.0152, 16.2235, 19.4094]}

VOL_DT = mybir.dt.float8e3   # E3M4: native PE dtype, 1 cyc/row
W_DT = mybir.dt.float8e3     # basis dtype: e3m4, per-column scaled (host
# unscales); halves the basis bytes that compete with the first chunks
NWARM = 42                   # PE warmup matmuls (N=128) during DMA latency
NP_VOL_DT = ml_dtypes.float8_e3m4
TRACE = False                # test harness sets True for NTFF profiling
LAST_RESULTS = None          # BassKernelResults of the most recent run


def _bessel_j(n, x):
    xs = np.maximum(x, 1e-12)
    if n == 0:
        small = np.abs(x) < 1.0
        med = (np.abs(x) >= 1.0) & (np.abs(x) < 5.0)
        sm = 1.0 - x ** 2 / 4.0 + x ** 4 / 64.0
        md = np.cos(x - np.pi / 4) / np.sqrt(xs)
        lg = np.sqrt(2.0 / (np.pi * xs)) * np.cos(x - np.pi / 4)
        return np.where(small, sm, np.where(med, md, lg))
    elif n == 1:
        small = np.abs(x) < 1.0
        med = (np.abs(x) >= 1.0) & (np.abs(x) < 5.0)
        sm = x / 2.0 - x ** 3 / 16.0
        md = np.sin(x - np.pi / 4) / np.sqrt(xs)
        lg = np.sqrt(2.0 / (np.pi * xs)) * np.cos(x - 3 * np.pi / 4)
        return np.where(small, sm, np.where(med, md, lg))
    else:
        logfact = sum(math.log(i) for i in range(1, n + 1))
        small = np.abs(x) < 0.1 * n
        sm = np.exp(n * np.log(xs / 2.0) - logfact)
        lg = np.sqrt(2.0 / (np.pi * xs)) * np.cos(x - (2 * n + 1) * np.pi / 4)
        return np.where(small, sm, lg)


def _make_basis():
    """C1_perm [128, Q*NJ] f32 and ax_cat [Z, NL] f32; dtheta*dz in ax_cat."""
    r = np.linspace(0.0, 1.0, R) * R_SCALE
    theta = np.linspace(0.0, 2 * math.pi, T)
    z = np.linspace(-1.0, 1.0, Z)
    dr = R_SCALE / (R - 1)
    dtheta = 2 * math.pi / T
    dz = 2.0 / (Z - 1)
    Wm = np.zeros((4, MAX_K, R))
    for m in range(4):
        for k in range(1, MAX_K + 1):
            r_nk = BESSEL_ZEROS[m][k - 1]
            J = _bessel_j(m, r_nk * r)
            ss = (T * Z) * np.sum((J * r * dr) ** 2)
            norm = 1.0 / np.sqrt(ss) if ss > 1e-6 else 0.0
            Wm[m, k - 1] = J * norm * r * dr
    ang_scale = 1.0 / math.sqrt(2 * math.pi)
    C1 = np.zeros((RT, NJ))
    for m in range(4):
        cosm = np.cos(m * theta) * ang_scale
        sinm = np.sin(m * theta) * ang_scale
        for k in range(MAX_K):
            C1[:, m * 4 + k] = (Wm[m, k][:, None] * cosm[None, :]).reshape(-1)
            if m >= 1:
                C1[:, 16 + (m - 1) * 4 + k] = (
                    Wm[m, k][:, None] * sinm[None, :]).reshape(-1)
    # permute rows to the [128, 6912] data layout: K-tile j holds rt = p*Q + j
    C1_perm = C1.reshape(P, Q, NJ).reshape(P, Q * NJ)
    l_vals = np.arange(-MAX_L, MAX_L + 1)
    ax_scale = (1.0 / math.sqrt(2)) * dtheta * dz
    ax_cat = np.zeros((Z, NL))
    for li, lv in enumerate(l_vals):
        ax_cat[:, li] = np.cos(math.pi * lv * z) * ax_scale
        ax_cat[:, 11 + li] = np.sin(math.pi * lv * z) * ax_scale
    return (np.ascontiguousarray(C1_perm, dtype=np.float32),
            np.ascontiguousarray(ax_cat, dtype=np.float32))


def _combine(out2):
    """out2 [..., 28, 22] f32 -> ch [..., 7, 4, 11] complex64 (the +/-n unfold)."""
    lead = out2.shape[:-2]
    E = out2[..., :16, :].reshape(*lead, 4, MAX_K, 2, 11)  # cos block, q=0 re / 1 im
    O = out2[..., 16:, :].reshape(*lead, 3, MAX_K, 2, 11)  # sin block, m=1..3
    ch = np.zeros((*lead, 2 * MAX_N + 1, MAX_K, 2 * MAX_L + 1), dtype=np.complex64)
    ch[..., 3, :, :] = E[..., 0, :, 0, :] + 1j * E[..., 0, :, 1, :]
    for m in range(1, 4):
        Er, Ei = E[..., m, :, 0, :], E[..., m, :, 1, :]
        Or_, Oi = O[..., m - 1, :, 0, :], O[..., m - 1, :, 1, :]
        ch[..., 3 + m, :, :] = (Er - Oi) + 1j * (Ei + Or_)
        ch[..., 3 - m, :, :] = (Er + Oi) + 1j * (Ei - Or_)
    return ch


def _build_nc():
    f32 = mybir.dt.float32
    nc = bacc.Bacc("TRN2", target_bir_lowering=False, debug=False,
                   num_devices=N_CORES)
    vol_in = nc.dram_tensor("vol", [P, BC_PER_CORE * Q * Z], VOL_DT,
                            kind="ExternalInput")
    c1_in = nc.dram_tensor("c1", [P, NQUAD * P], W_DT, kind="ExternalInput")
    out = nc.dram_tensor("out", [P, NGRP * GRP * Z], mybir.dt.bfloat16,
                         kind="ExternalOutput")

    with tile.TileContext(nc) as tc:
        with (
            tc.tile_pool(name="consts", bufs=1) as consts,
            tc.tile_pool(name="vpool", bufs=5) as vpool,
            tc.tile_pool(name="obuf", bufs=2) as obuf,
            tc.tile_pool(name="pspool", bufs=1, space="PSUM") as pspool,
        ):
            # basis head (weights for the first chunk's K-tiles) rides the
            # sync ring FIRST: it is tiny, so the first matmul is gated only
            # by the first volume chunk.  The scalar ring pays a ~1.3 us
            # ACT_TABLE_LOAD before its first instruction, so it only gets
            # late chunks + the second output copy.
            c1_sb = consts.tile([P, NQUAD * P], W_DT)
            hq = (C1_HEAD // 4) * P
            nc.sync.dma_start(c1_sb[:, :hq], c1_in[:, :hq])
            # rest of the basis on the scalar ring: flows concurrently with
            # the early chunks, lands well before K-tile C1_HEAD is due
            nc.scalar.dma_start(c1_sb[:, hq:], c1_in[:, hq:])
            # PE warmup: ~40 tiny matmuls on a zeroed tile keep the PE busy
            # during the DMA-latency window so HAM un-throttles (K=8/8)
            # before the first real matmul; results land in a scratch bank.
            wz = consts.tile([P, 512], VOL_DT)
            nc.vector.memset(wz[:], 0)
            psw = pspool.tile([NJ, 128], f32, name="psw")
            for _ in range(NWARM):
                nc.tensor.matmul(psw[:], wz[:, :NJ], wz[:, 128:256],
                                 start=True, stop=True)
            # one full [128, 384] accumulator per bc-group: col-group cg
            # holds partials for js == cg (mod 4) at partitions 32cg..32cg+27
            acc = [pspool.tile([P, GRP * Z], f32, name=f"acc{g}")
                   for g in range(NGRP)]
            j0 = 0
            off = 0
            for ci, jchunk in enumerate(CHUNKS):
                cb = BC_PER_CORE * jchunk * Z
                v8 = vpool.tile([P, BC_PER_CORE * max(CHUNKS) * Z], VOL_DT,
                                padded_shape=[P, BC_PER_CORE * max(CHUNKS) * Z])
                # early chunks on sync (behind the tiny c1 head); late
                # chunks ride the scalar ring with the basis rest
                eng = nc.sync if ci % 2 == 0 else nc.scalar
                eng.dma_start(v8[:, :cb], vol_in[:, off:off + cb])
                off += cb
                v8r = v8[:, :cb].rearrange(
                    "p (b j z) -> p b j z", b=BC_PER_CORE, j=jchunk)
                for jj in range(jchunk):
                    j = j0 + jj
                    q, cg = divmod(j, 4)
                    for g in range(NGRP):
                        # col-tiled: consecutive js occupy different 32-col
                        # groups of the array and run concurrently
                        nc.tensor.matmul(
                            acc[g][32 * cg:32 * cg + NJ, :],
                            c1_sb[:, q * P + 32 * cg:q * P + 32 * cg + NJ],
                            v8r[:, g * GRP:(g + 1) * GRP, jj, :],
                            start=(q == 0),
                            stop=(q == NQUAD - 1),
                            tile_position=(0, 32 * cg),
                            skip_group_check=True,
                        )
                j0 += jchunk
            # full accumulators out; the 4-way col-group add happens on host
            ob = obuf.tile([P, NGRP * GRP * Z], mybir.dt.bfloat16)
            nc.vector.tensor_copy(ob[:, :GRP * Z], acc[0][:])
            nc.scalar.copy(ob[:, GRP * Z:], acc[1][:])
            nc.sync.dma_start(out[:], ob[:])

    nc.compile()
    return nc


_NC_CACHE = None


def _get_nc():
    global _NC_CACHE
    if _NC_CACHE is None:
        _NC_CACHE = _build_nc()
    return _NC_CACHE


def kernel(cylindrical_volume):
    global LAST_RESULTS
    vol = np.asarray(cylindrical_volume, dtype=np.float32)
    assert vol.shape == (B, C, R, T, Z), vol.shape
    c1_perm, ax_cat = _make_basis()
    # basis -> e3m4 with per-output-column scale (values are ~1e-4, far
    # below e3m4's normal range; scale to ~8 and unscale on host)
    C1 = c1_perm.reshape(P, Q, NJ)
    lam = 8.0 / np.abs(C1).max(axis=(0, 1))               # [NJ]
    c1q = np.zeros((P, Q // 4, 4, 32), dtype=NP_VOL_DT)
    c1q[:, :, :, :NJ] = (C1 * lam).reshape(P, Q // 4, 4, NJ).astype(NP_VOL_DT)
    c1_dev = np.ascontiguousarray(c1q.reshape(P, (Q // 4) * P))
    # volume -> e3m4 with first-order noise shaping along z: the host
    # stage-2 projects z onto |l|<=5 of 96 modes, so (1-z^-1)-shaped
    # quantization noise is filtered ~3x (rel err 1.4e-2 -> 4e-3 at fp16
    # basis; 1.2e-2 with the e3m4 basis)
    vr = np.ascontiguousarray(vol).reshape(BC, P, Q, Z)
    volq = np.empty((BC, P, Q, Z), dtype=NP_VOL_DT)
    err = np.zeros((BC, P, Q), dtype=np.float32)
    for k in range(Z):
        acc = vr[..., k] + err
        qk = acc.astype(NP_VOL_DT)
        volq[..., k] = qk
        err = acc - qk.astype(np.float32)

    nc = _get_nc()
    in_maps = []
    for i in range(N_CORES):
        vc = volq[i * BC_PER_CORE:(i + 1) * BC_PER_CORE]  # [8, 128, 72, 96]
        vt = vc.transpose(1, 0, 2, 3)                     # [128, 8, 72, 96]
        j0 = 0
        blocks = []
        for jc in CHUNKS:
            blocks.append(vt[:, :, j0:j0 + jc, :].reshape(P, -1))
            j0 += jc
        in_maps.append({"vol": np.ascontiguousarray(np.concatenate(blocks, axis=1)),
                        "c1": c1_dev})
    import os
    try:
        res = run_bass_kernel_spmd(nc, in_maps, list(range(N_CORES)),
                                   trace=TRACE)
    except ModuleNotFoundError:
        # BASS_TRACE set but this image lacks the axon NTFF hook module;
        # rerun without tracing rather than failing
        os.environ["BASS_NEVER_TRACE"] = "1"
        try:
            res = run_bass_kernel_spmd(nc, in_maps, list(range(N_CORES)),
                                       trace=False)
        finally:
            os.environ.pop("BASS_NEVER_TRACE", None)
    LAST_RESULTS = res
    # per-core out [128, NGRP*GRP*96]: sum the 4 col-group partials
    # (rows 32cg+c) -> [8bc, 28, 96z]
    S = np.concatenate(
        [res.results[i]["out"].astype(np.float32)
         .reshape(4, 32, NGRP, GRP, Z)[:, :NJ].sum(axis=0)
         .transpose(1, 2, 0, 3).reshape(BC_PER_CORE, NJ, Z)
         for i in range(N_CORES)], axis=0)          # [64, 28, 96]
    S /= lam[None, :, None]                          # undo basis column scale
    out2 = np.einsum('bjz,zl->bjl', S, ax_cat)       # host stage 2: [64, 28, 22]
    ch = _combine(out2)
    return ch.reshape(B, C, 2 * MAX_N + 1, MAX_K, 2 * MAX_L + 1)


# revision 48
# speedup vs baseline: 1.1728x; 1.0964x over previous
"""Trainium2 Bass kernel for nn_CHTransform (cylindrical-harmonics decomposition).

Math: ch[b,c,n,k,l] = dtheta*dz * sum_{r,t,z} vol[b,c,r,t,z]
                       * Wr[|n|,k,r] * e^{i n theta_t}/sqrt(2pi) * e^{i pi l z_z}/sqrt(2)

The angular basis is even (cos) / odd (sin) in n and the radial basis depends
only on |n|, so only m=|n| in 0..3 is needed: a combined host-precomputed basis
C1[rt, j] (16 cos-cols (m,k) + 12 sin-cols (m>=1,k), 28 total) contracts r and
t in one TensorE pass; the tiny z-contraction against the axial basis and the
+/-n complex unfold happen on host during the unshard (64 x 28 x 96 floats).

Precision: the volume is host-converted to fp8 E3M4 (native PE dtype, 1 B/elt,
4 mantissa bits) with FIRST-ORDER NOISE SHAPING along z: the host stage-2
projects z onto |l|<=5 of 96 modes, so (1-z^-1)-shaped quantization noise is
attenuated ~3x by the projection (worst mode keeps 2*sin(pi*5/96) = 0.33).
The basis is e3m4 too (per-column scaled by ~8/max into the normal range,
unscaled on host).  Measured end-to-end rel err 1.214e-2 < 2e-2 gate.  This
quarters HBM traffic vs the fp32 baseline (27 -> 6.75 MiB/core), moving the
bottleneck to the PE itself: 55296 moving rows @ 1 cyc/row @ 2.4 GHz = 23 us.

Device (per core: 8 of the 64 (b,c) pairs, data-parallel, no communication):
  - vol arrives as [8, 128, 6912] e3m4: partition p holds 72 consecutive
    rt-rows; K-tile j of the contraction lives at free columns j*96..(j+1)*96,
    i.e. rt = p*72 + j, with C1 host-permuted to match.
  - (b,c) are processed in 2 groups of 4.  The 28-col stationary uses only
    28/128 PE columns, so K-tiles are COL-TILED: quad q's four js occupy
    the four 32-col groups (tile_position=(0,32cg)) and their matmuls run
    CONCURRENTLY in the array (~3.7x stream speedup, PE now sub-critical).
    Each col-group accumulates js == cg (mod 4) into partition slice
    [32cg:32cg+28] of a full [128, 384] PSUM bank per group; the 4-way
    cross-partition add happens on host (output is [128, 768] bf16).
  - host pre-arranges the volume chunk-major so every chunk DMA is one
    fully-contiguous [128, 8*jc*96] transfer (128 descriptors, >=4.6 KB
    each); early chunks ride the sync HWDGE ring behind a tiny basis head,
    late chunks + basis rest ride the scalar ring concurrently.
  - 42 N=128 warmup matmuls on a zeroed tile span the DMA-latency window
    so the HAM clock gate is at K=8/8 when real matmuls start.
  - with the PE sub-critical the kernel is DMA-bound end-to-end: chunks
    are front-loaded big ([16,16,16,12,8,4]) for maximum flow rate, the
    last chunk small so its completion sem (+~2.9 us receipt lag) lands
    right after flow-end.

Measured: 34.7-34.8 us (full clock) on 8 cores vs 104.5 us fp32 baseline:
~7 us framework preamble, ~18 us HBM flow @ ~400 GB/s, last-chunk sem
lag, ~4 us tail.  Exec time has +-1-2 us run jitter and occasional
chip-wide slow phases (PE at 2.0 GHz, DMA slower too -> ~41 us).
"""

import math

import numpy as np
import ml_dtypes

import concourse.bacc as bacc
import concourse.mybir as mybir
import concourse.tile as tile
from concourse.bass_utils import run_bass_kernel_spmd

# Problem constants (hardcoded per spec nn_CHTransform_43439299231904)
B, C, R, T, Z = 8, 8, 96, 96, 96
MAX_N, MAX_K, MAX_L = 3, 4, 5
R_SCALE = 1.0
N_CORES = 8
BC = B * C                   # 64 (b,c) pairs
BC_PER_CORE = BC // N_CORES  # 8
RT = R * T                   # 9216
P = 128                      # SBUF partitions
Q = RT // P                  # 72 rt-rows per partition = # of K-tiles
NJ = 28                      # stage-1 output columns: 16 cos (m,k) + 12 sin
NL = 22                      # host stage-2 columns: 11 cos l + 11 sin l
GRP = 4                      # (b,c) pairs per matmul group (N = GRP*Z = 384)
NGRP = BC_PER_CORE // GRP    # 2
CHUNKS = [12, 16, 12, 12, 8, 8, 2, 2]  # K-tiles per DMA chunk (all 8 bc per chunk;
# chunk boundaries are multiples of 4 so j-quads never straddle chunks).
# Host pre-arranges the volume chunk-major: each chunk is one contiguous
# [128, 8*jc*96] transfer.
C1_HEAD = 12                 # K-tiles of basis in the head (sync-ring) DMA
NQUAD = Q // 4               # 18 j-quads; quad q covers js 4q..4q+3, one
# per 32-col group of the PE array (col-tiled concurrent matmuls)

BESSEL_ZEROS = {0: [2.4048, 5.5201, 8.6537, 11.7915, 14.9309],
                1: [3.8317, 7.0156, 10.1735, 13.3237, 16.4706],
                2: [5.1356, 8.4172, 11.6198, 14.796, 18.0155],
                3: [6.3802, 9.761, 13.0152, 16.2235, 19.4094]}

VOL_DT = mybir.dt.float8e3   # E3M4: native PE dtype, 1 cyc/row
W_DT = mybir.dt.float8e3     # basis dtype: e3m4, per-column scaled (host
# unscales); halves the basis bytes that compete with the first chunks
NWARM = 42                   # PE warmup matmuls (N=128) during DMA latency
NP_VOL_DT = ml_dtypes.float8_e3m4
TRACE = False                # test harness sets True for NTFF profiling
LAST_RESULTS = None          # BassKernelResults of the most recent run


def _bessel_j(n, x):
    xs = np.maximum(x, 1e-12)
    if n == 0:
        small = np.abs(x) < 1.0
        med = (np.abs(x) >= 1.0) & (np.abs(x) < 5.0)
        sm = 1.0 - x ** 2 / 4.0 + x ** 4 / 64.0
        md = np.cos(x - np.pi / 4) / np.sqrt(xs)
        lg = np.sqrt(2.0 / (np.pi * xs)) * np.cos(x - np.pi / 4)
        return np.where(small, sm, np.where(med, md, lg))
    elif n == 1:
        small = np.abs(x) < 1.0
        med = (np.abs(x) >= 1.0) & (np.abs(x) < 5.0)
        sm = x / 2.0 - x ** 3 / 16.0
        md = np.sin(x - np.pi / 4) / np.sqrt(xs)
        lg = np.sqrt(2.0 / (np.pi * xs)) * np.cos(x - 3 * np.pi / 4)
        return np.where(small, sm, np.where(med, md, lg))
    else:
        logfact = sum(math.log(i) for i in range(1, n + 1))
        small = np.abs(x) < 0.1 * n
        sm = np.exp(n * np.log(xs / 2.0) - logfact)
        lg = np.sqrt(2.0 / (np.pi * xs)) * np.cos(x - (2 * n + 1) * np.pi / 4)
        return np.where(small, sm, lg)


def _make_basis():
    """C1_perm [128, Q*NJ] f32 and ax_cat [Z, NL] f32; dtheta*dz in ax_cat."""
    r = np.linspace(0.0, 1.0, R) * R_SCALE
    theta = np.linspace(0.0, 2 * math.pi, T)
    z = np.linspace(-1.0, 1.0, Z)
    dr = R_SCALE / (R - 1)
    dtheta = 2 * math.pi / T
    dz = 2.0 / (Z - 1)
    Wm = np.zeros((4, MAX_K, R))
    for m in range(4):
        for k in range(1, MAX_K + 1):
            r_nk = BESSEL_ZEROS[m][k - 1]
            J = _bessel_j(m, r_nk * r)
            ss = (T * Z) * np.sum((J * r * dr) ** 2)
            norm = 1.0 / np.sqrt(ss) if ss > 1e-6 else 0.0
            Wm[m, k - 1] = J * norm * r * dr
    ang_scale = 1.0 / math.sqrt(2 * math.pi)
    C1 = np.zeros((RT, NJ))
    for m in range(4):
        cosm = np.cos(m * theta) * ang_scale
        sinm = np.sin(m * theta) * ang_scale
        for k in range(MAX_K):
            C1[:, m * 4 + k] = (Wm[m, k][:, None] * cosm[None, :]).reshape(-1)
            if m >= 1:
                C1[:, 16 + (m - 1) * 4 + k] = (
                    Wm[m, k][:, None] * sinm[None, :]).reshape(-1)
    # permute rows to the [128, 6912] data layout: K-tile j holds rt = p*Q + j
    C1_perm = C1.reshape(P, Q, NJ).reshape(P, Q * NJ)
    l_vals = np.arange(-MAX_L, MAX_L + 1)
    ax_scale = (1.0 / math.sqrt(2)) * dtheta * dz
    ax_cat = np.zeros((Z, NL))
    for li, lv in enumerate(l_vals):
        ax_cat[:, li] = np.cos(math.pi * lv * z) * ax_scale
        ax_cat[:, 11 + li] = np.sin(math.pi * lv * z) * ax_scale
    return (np.ascontiguousarray(C1_perm, dtype=np.float32),
            np.ascontiguousarray(ax_cat, dtype=np.float32))


def _combine(out2):
    """out2 [..., 28, 22] f32 -> ch [..., 7, 4, 11] complex64 (the +/-n unfold)."""
    lead = out2.shape[:-2]
    E = out2[..., :16, :].reshape(*lead, 4, MAX_K, 2, 11)  # cos block, q=0 re / 1 im
    O = out2[..., 16:, :].reshape(*lead, 3, MAX_K, 2, 11)  # sin block, m=1..3
    ch = np.zeros((*lead, 2 * MAX_N + 1, MAX_K, 2 * MAX_L + 1), dtype=np.complex64)
    ch[..., 3, :, :] = E[..., 0, :, 0, :] + 1j * E[..., 0, :, 1, :]
    for m in range(1, 4):
        Er, Ei = E[..., m, :, 0, :], E[..., m, :, 1, :]
        Or_, Oi = O[..., m - 1, :, 0, :], O[..., m - 1, :, 1, :]
        ch[..., 3 + m, :, :] = (Er - Oi) + 1j * (Ei + Or_)
        ch[..., 3 - m, :, :] = (Er + Oi) + 1j * (Ei - Or_)
    return ch


def _build_nc():
    f32 = mybir.dt.float32
    nc = bacc.Bacc("TRN2", target_bir_lowering=False, debug=False,
                   num_devices=N_CORES)
    vol_in = nc.dram_tensor("vol", [P, BC_PER_CORE * Q * Z], VOL_DT,
                            kind="ExternalInput")
    c1_in = nc.dram_tensor("c1", [P, NQUAD * P], W_DT, kind="ExternalInput")
    out = nc.dram_tensor("out", [P, NGRP * GRP * Z], mybir.dt.bfloat16,
                         kind="ExternalOutput")

    with tile.TileContext(nc) as tc:
        with (
            tc.tile_pool(name="consts", bufs=1) as consts,
            tc.tile_pool(name="vpool", bufs=5) as vpool,
            tc.tile_pool(name="obuf", bufs=2) as obuf,
            tc.tile_pool(name="pspool", bufs=1, space="PSUM") as pspool,
        ):
            # basis head (weights for the first chunk's K-tiles) rides the
            # sync ring FIRST: it is tiny, so the first matmul is gated only
            # by the first volume chunk.  The scalar ring pays a ~1.3 us
            # ACT_TABLE_LOAD before its first instruction, so it only gets
            # late chunks + the second output copy.
            c1_sb = consts.tile([P, NQUAD * P], W_DT)
            hq = (C1_HEAD // 4) * P
            nc.sync.dma_start(c1_sb[:, :hq], c1_in[:, :hq])
            # rest of the basis on the scalar ring: flows concurrently with
            # the early chunks, lands well before K-tile C1_HEAD is due
            nc.scalar.dma_start(c1_sb[:, hq:], c1_in[:, hq:])
            # PE warmup: ~40 tiny matmuls on a zeroed tile keep the PE busy
            # during the DMA-latency window so HAM un-throttles (K=8/8)
            # before the first real matmul; results land in a scratch bank.
            wz = consts.tile([P, 512], VOL_DT)
            nc.vector.memset(wz[:], 0)
            psw = pspool.tile([NJ, 128], f32, name="psw")
            for _ in range(NWARM):
                nc.tensor.matmul(psw[:], wz[:, :NJ], wz[:, 128:256],
                                 start=True, stop=True)
            # one full [128, 384] accumulator per bc-group: col-group cg
            # holds partials for js == cg (mod 4) at partitions 32cg..32cg+27
            acc = [pspool.tile([P, GRP * Z], f32, name=f"acc{g}")
                   for g in range(NGRP)]
            ob = obuf.tile([P, NGRP * GRP * Z], mybir.dt.bfloat16)
            j0 = 0
            off = 0
            for ci, jchunk in enumerate(CHUNKS):
                cb = BC_PER_CORE * jchunk * Z
                v8 = vpool.tile([P, BC_PER_CORE * max(CHUNKS) * Z], VOL_DT,
                                padded_shape=[P, BC_PER_CORE * max(CHUNKS) * Z])
                # early chunks on sync (behind the tiny c1 head); late
                # chunks ride the scalar ring with the basis rest
                eng = nc.sync if ci % 2 == 0 else nc.scalar
                eng.dma_start(v8[:, :cb], vol_in[:, off:off + cb])
                off += cb
                v8r = v8[:, :cb].rearrange(
                    "p (b j z) -> p b j z", b=BC_PER_CORE, j=jchunk)
                for jj in range(jchunk):
                    j = j0 + jj
                    q, cg = divmod(j, 4)
                    for g in range(NGRP):
                        # col-tiled: consecutive js occupy different 32-col
                        # groups of the array and run concurrently
                        nc.tensor.matmul(
                            acc[g][32 * cg:32 * cg + NJ, :],
                            c1_sb[:, q * P + 32 * cg:q * P + 32 * cg + NJ],
                            v8r[:, g * GRP:(g + 1) * GRP, jj, :],
                            start=(q == 0),
                            stop=(q == NQUAD - 1),
                            tile_position=(0, 32 * cg),
                            skip_group_check=True,
                        )
                j0 += jchunk
                if j0 == Q - 2:
                    # cg 0/1 fully accumulated: copy + store partitions
                    # [0:64] while the last chunk's sem wait elapses
                    nc.vector.tensor_copy(ob[:64, :GRP * Z], acc[0][:64, :])
                    nc.scalar.copy(ob[:64, GRP * Z:], acc[1][:64, :])
                    nc.sync.dma_start(out[:64, :], ob[:64, :])
            nc.vector.tensor_copy(ob[64:, :GRP * Z], acc[0][64:, :])
            nc.scalar.copy(ob[64:, GRP * Z:], acc[1][64:, :])
            nc.sync.dma_start(out[64:, :], ob[64:, :])

    nc.compile()
    return nc


_NC_CACHE = None


def _get_nc():
    global _NC_CACHE
    if _NC_CACHE is None:
        _NC_CACHE = _build_nc()
    return _NC_CACHE


def kernel(cylindrical_volume):
    global LAST_RESULTS
    vol = np.asarray(cylindrical_volume, dtype=np.float32)
    assert vol.shape == (B, C, R, T, Z), vol.shape
    c1_perm, ax_cat = _make_basis()
    # basis -> e3m4 with per-output-column scale (values are ~1e-4, far
    # below e3m4's normal range; scale to ~8 and unscale on host)
    C1 = c1_perm.reshape(P, Q, NJ)
    lam = 8.0 / np.abs(C1).max(axis=(0, 1))               # [NJ]
    c1q = np.zeros((P, Q // 4, 4, 32), dtype=NP_VOL_DT)
    c1q[:, :, :, :NJ] = (C1 * lam).reshape(P, Q // 4, 4, NJ).astype(NP_VOL_DT)
    c1_dev = np.ascontiguousarray(c1q.reshape(P, (Q // 4) * P))
    # volume -> e3m4 with first-order noise shaping along z: the host
    # stage-2 projects z onto |l|<=5 of 96 modes, so (1-z^-1)-shaped
    # quantization noise is filtered ~3x (rel err 1.4e-2 -> 4e-3 at fp16
    # basis; 1.2e-2 with the e3m4 basis)
    vr = np.ascontiguousarray(vol).reshape(BC, P, Q, Z)
    volq = np.empty((BC, P, Q, Z), dtype=NP_VOL_DT)
    err = np.zeros((BC, P, Q), dtype=np.float32)
    for k in range(Z):
        acc = vr[..., k] + err
        qk = acc.astype(NP_VOL_DT)
        volq[..., k] = qk
        err = acc - qk.astype(np.float32)

    nc = _get_nc()
    in_maps = []
    for i in range(N_CORES):
        vc = volq[i * BC_PER_CORE:(i + 1) * BC_PER_CORE]  # [8, 128, 72, 96]
        vt = vc.transpose(1, 0, 2, 3)                     # [128, 8, 72, 96]
        j0 = 0
        blocks = []
        for jc in CHUNKS:
            blocks.append(vt[:, :, j0:j0 + jc, :].reshape(P, -1))
            j0 += jc
        in_maps.append({"vol": np.ascontiguousarray(np.concatenate(blocks, axis=1)),
                        "c1": c1_dev})
    import os
    try:
        res = run_bass_kernel_spmd(nc, in_maps, list(range(N_CORES)),
                                   trace=TRACE)
    except ModuleNotFoundError:
        # BASS_TRACE set but this image lacks the axon NTFF hook module;
        # rerun without tracing rather than failing
        os.environ["BASS_NEVER_TRACE"] = "1"
        try:
            res = run_bass_kernel_spmd(nc, in_maps, list(range(N_CORES)),
                                       trace=False)
        finally:
            os.environ.pop("BASS_NEVER_TRACE", None)
    LAST_RESULTS = res
    # per-core out [128, NGRP*GRP*96]: sum the 4 col-group partials
    # (rows 32cg+c) -> [8bc, 28, 96z]
    S = np.concatenate(
        [res.results[i]["out"].astype(np.float32)
         .reshape(4, 32, NGRP, GRP, Z)[:, :NJ].sum(axis=0)
         .transpose(1, 2, 0, 3).reshape(BC_PER_CORE, NJ, Z)
         for i in range(N_CORES)], axis=0)          # [64, 28, 96]
    S /= lam[None, :, None]                          # undo basis column scale
    out2 = np.einsum('bjz,zl->bjl', S, ax_cat)       # host stage 2: [64, 28, 22]
    ch = _combine(out2)
    return ch.reshape(B, C, 2 * MAX_N + 1, MAX_K, 2 * MAX_L + 1)
